# revision 2
# baseline (speedup 1.0000x reference)
"""ArcFace loss distributed Bass kernel for 8 TRN2 NeuronCores.

Strategy (class-parallel / tensor-parallel over the 100000-class dim):
  - Host: pad classes 100000 -> 8*12544, transpose W shard to [D, C_shard] per core,
    gather W[target] rows (pure data movement; no arithmetic on host).
  - Device (SPMD, identical program on 8 cores):
      * normalize x rows, transpose to xnT [D, B] (PE transpose)
      * stream WT tiles; f32r matmul cosT[c,b] = WT.T @ xnT (classes on partitions)
      * per-class ||w||^2 via bf16 gram matmul diag (diag extracted with one
        scalar_tensor_tensor against an identity mask, accum_out -> [c,1])
      * exp fused on ScalarE: E = exp(S*rsqrt(ssq)*P - 30) with per-partition
        scale AP (rsqrt computed as exp(-0.5*ln(ssq)+ln(S)) to stay in the
        natural_log_exp table set)
      * partition-sum of E via ones-vector matmul accumulating into one PSUM row
      * target-logit correction computed densely for all 512 rows on every core
        from host-gathered W[target] (row-dots on VectorE, margin math, tiny)
      * AllReduce the [1,512] partial sums across the 8 cores, then
        nll = 30 + ln(total) - S*phi, mean -> scalar
Fixed max shift of S=30 is used for the softmax (cos <= 1), so no running max
is needed: exp(S*cos - 30) is exact and never overflows/underflows in fp32.
"""

import math
from contextlib import ExitStack

import numpy as np

from concourse import bacc, masks, mybir, tile
from concourse.bass_utils import run_bass_kernel_spmd

N_CORES = 8
B = 512
D = 512
NCLASS = 100000
C_SHARD = NCLASS // N_CORES      # 12500
C_PAD = 12544                    # 98 * 128
C_TILE = 1792                    # 7 macro tiles of 14 subchunks
S = 30.0
MARGIN = 0.5
COS_M = math.cos(MARGIN)
SIN_M = math.sin(MARGIN)
BIAS = -30.0                     # fixed log-sum-exp shift (= -S)

f32 = mybir.dt.float32
f32r = mybir.dt.float32r
bf16 = mybir.dt.bfloat16
AF = mybir.ActivationFunctionType
ALU = mybir.AluOpType
AX = mybir.AxisListType

P = 128


def build_arcface_nc(c_pad=C_PAD, c_tile=C_TILE, c_real=C_SHARD, n_cores=N_CORES):
    assert c_pad % c_tile == 0 and c_tile % P == 0
    n_tiles = c_pad // c_tile
    n_sub = c_tile // P
    n_dk = D // P
    n_bk = B // P
    pad_corr = float(n_cores * (c_pad - c_real)) * math.exp(BIAS)

    nc = bacc.Bacc("TRN2", target_bir_lowering=False, debug=False,
                   num_devices=n_cores)

    wt_ext = nc.dram_tensor("wt", [D, c_pad], f32, kind="ExternalInput")
    x_ext = nc.dram_tensor("x", [B, D], f32, kind="ExternalInput")
    wtg_ext = nc.dram_tensor("wtg", [B, D], f32, kind="ExternalInput")
    out_ext = nc.dram_tensor("out", [1, 1], f32, kind="ExternalOutput")

    with ExitStack() as ctx:
        tc = ctx.enter_context(tile.TileContext(nc))
        cpool = ctx.enter_context(tc.tile_pool(name="consts", bufs=1))
        xpool = ctx.enter_context(tc.tile_pool(name="xpool", bufs=1))
        sm = ctx.enter_context(tc.tile_pool(name="smalls", bufs=1))
        spool = ctx.enter_context(tc.tile_pool(name="spool", bufs=2))
        wtpool = ctx.enter_context(tc.tile_pool(name="wtpool", bufs=2))
        wbpool = ctx.enter_context(tc.tile_pool(name="wbpool", bufs=2))
        epool = ctx.enter_context(tc.tile_pool(name="epool", bufs=4))
        jpool = ctx.enter_context(tc.tile_pool(name="jpool", bufs=2))
        ps_c = ctx.enter_context(tc.tile_pool(name="ps_c", bufs=3, space="PSUM"))
        ps_g = ctx.enter_context(tc.tile_pool(name="ps_g", bufs=2, space="PSUM"))
        ps_acc = ctx.enter_context(tc.tile_pool(name="ps_acc", bufs=1, space="PSUM"))
        dram = ctx.enter_context(tc.tile_pool(name="dram", bufs=1, space="DRAM"))

        # ---- constants ----
        ident = cpool.tile([P, P], f32)
        masks.make_identity(nc, ident[:])
        ones_bf = cpool.tile([P, 1], bf16)
        nc.vector.memset(ones_bf[:], 1.0)
        ones_f = cpool.tile([P, 1], f32)
        nc.vector.memset(ones_f[:], 1.0)
        bias_m30 = cpool.tile([P, 1], f32)
        nc.vector.memset(bias_m30[:], BIAS)
        bias_lnS = cpool.tile([P, 1], f32)
        nc.vector.memset(bias_lnS[:], float(np.log(S)))

        # ---- x prep: load, row-norms, normalize, transpose ----
        xb = [xpool.tile([P, D], f32, name=f"xb{k}") for k in range(n_bk)]
        for k in range(n_bk):
            nc.sync.dma_start(out=xb[k][:], in_=x_ext.ap()[k * P:(k + 1) * P, :])
        qx = sm.tile([P, n_bk], f32)
        junk_a = jpool.tile([P, D], f32, tag="junk")
        for k in range(n_bk):
            nc.vector.scalar_tensor_tensor(
                out=junk_a[:], in0=xb[k][:], scalar=1.0, in1=xb[k][:],
                op0=ALU.mult, op1=ALU.mult, accum_out=qx[:, k:k + 1])
        rx = sm.tile([P, n_bk], f32)
        nc.scalar.activation(rx[:], qx[:], AF.Ln)
        nc.scalar.activation(rx[:], rx[:], AF.Exp, scale=-0.5)  # rsqrt(qx)

        xn = [xpool.tile([P, D], f32, name=f"xn{k}") for k in range(n_bk)]
        for k in range(n_bk):
            nc.vector.tensor_scalar(out=xn[k][:], in0=xb[k][:],
                                    scalar1=rx[:, k:k + 1], scalar2=None,
                                    op0=ALU.mult)
        xnt = [xpool.tile([P, B], f32r, name=f"xnt{d}") for d in range(n_dk)]
        for k in range(n_bk):
            tp_ps = ps_c.tile([P, B], f32, tag="cos", name=f"tp_ps{k}")
            for d in range(n_dk):
                nc.tensor.transpose(tp_ps[:, d * P:(d + 1) * P],
                                    xn[k][:, d * P:(d + 1) * P], ident[:])
            for d in range(n_dk):
                nc.vector.tensor_copy(xnt[d][:, k * P:(k + 1) * P],
                                      tp_ps[:, d * P:(d + 1) * P])

        # ---- target margin terms (dense over all B rows, every core) ----
        wg = [xpool.tile([P, D], f32, name=f"wg{k}") for k in range(n_bk)]
        for k in range(n_bk):
            nc.sync.dma_start(out=wg[k][:], in_=wtg_ext.ap()[k * P:(k + 1) * P, :])
        qw = sm.tile([P, n_bk], f32)
        pt = sm.tile([P, n_bk], f32)
        junk_b = jpool.tile([P, D], f32, tag="junk")
        for k in range(n_bk):
            nc.vector.scalar_tensor_tensor(
                out=junk_b[:], in0=wg[k][:], scalar=1.0, in1=wg[k][:],
                op0=ALU.mult, op1=ALU.mult, accum_out=qw[:, k:k + 1])
            nc.vector.scalar_tensor_tensor(
                out=junk_b[:], in0=xb[k][:], scalar=1.0, in1=wg[k][:],
                op0=ALU.mult, op1=ALU.mult, accum_out=pt[:, k:k + 1])
        q = sm.tile([P, n_bk], f32)
        nc.vector.tensor_mul(q[:], qw[:], qx[:])
        nc.vector.tensor_scalar(out=q[:], in0=q[:], scalar1=1e-30, scalar2=None,
                                op0=ALU.max)
        rq = sm.tile([P, n_bk], f32)
        nc.scalar.activation(rq[:], q[:], AF.Ln)
        nc.scalar.activation(rq[:], rq[:], AF.Exp, scale=-0.5)
        cos_t = sm.tile([P, n_bk], f32)
        nc.vector.tensor_mul(cos_t[:], pt[:], rq[:])
        # sine = sqrt(max(1 - cos^2, eps))
        om = sm.tile([P, n_bk], f32)
        nc.vector.tensor_mul(om[:], cos_t[:], cos_t[:])
        nc.vector.tensor_scalar(out=om[:], in0=om[:], scalar1=-1.0, scalar2=1.0,
                                op0=ALU.mult, op1=ALU.add)
        nc.vector.tensor_scalar(out=om[:], in0=om[:], scalar1=1e-36, scalar2=None,
                                op0=ALU.max)
        sine = sm.tile([P, n_bk], f32)
        nc.scalar.activation(sine[:], om[:], AF.Ln)
        nc.scalar.activation(sine[:], sine[:], AF.Exp, scale=0.5)
        # phi = cos*COS_M - sine*SIN_M ; easy margin: cos>0 ? phi : cos
        tmp = sm.tile([P, n_bk], f32)
        nc.vector.tensor_scalar(out=tmp[:], in0=cos_t[:], scalar1=COS_M,
                                scalar2=None, op0=ALU.mult)
        phi = sm.tile([P, n_bk], f32)
        nc.vector.scalar_tensor_tensor(out=phi[:], in0=sine[:], scalar=-SIN_M,
                                       in1=tmp[:], op0=ALU.mult, op1=ALU.add)
        mask = sm.tile([P, n_bk], mybir.dt.uint8)
        nc.vector.tensor_scalar(out=mask[:], in0=cos_t[:], scalar1=0.0,
                                scalar2=None, op0=ALU.is_gt)
        phi_f = sm.tile([P, n_bk], f32)
        nc.vector.select(phi_f[:], mask[:], phi[:], cos_t[:])
        # delta = exp(S*phi_f - 30) - exp(S*cos_t - 30)
        e1 = sm.tile([P, n_bk], f32)
        nc.scalar.activation(e1[:], phi_f[:], AF.Exp, bias=bias_m30[:], scale=S)
        e2 = sm.tile([P, n_bk], f32)
        nc.scalar.activation(e2[:], cos_t[:], AF.Exp, bias=bias_m30[:], scale=S)
        delta = sm.tile([P, n_bk], f32)
        nc.vector.tensor_sub(delta[:], e1[:], e2[:])

        # ---- main loop over class tiles ----
        sumE_ps = ps_acc.tile([1, B], f32)
        for t in range(n_tiles):
            wtt = [wtpool.tile([P, c_tile], f32r, name=f"wtt{d}", tag=f"wtt{d}")
                   for d in range(n_dk)]
            for d in range(n_dk):
                nc.sync.dma_start(
                    out=wtt[d][:],
                    in_=wt_ext.ap()[d * P:(d + 1) * P,
                                    t * c_tile:(t + 1) * c_tile].bitcast(f32r))
            wtb = [wbpool.tile([P, c_tile], bf16, name=f"wtb{d}", tag=f"wtb{d}")
                   for d in range(n_dk)]
            for d in range(n_dk):
                nc.vector.tensor_copy(wtb[d][:], wtt[d][:].bitcast(f32))

            ssq = spool.tile([P, n_sub], f32, name="ssq")
            for s in range(n_sub):
                g_ps = ps_g.tile([P, P], f32, tag="g", name="g_ps")
                for d in range(n_dk):
                    nc.tensor.matmul(g_ps[:], wtb[d][:, s * P:(s + 1) * P],
                                     wtb[d][:, s * P:(s + 1) * P],
                                     start=(d == 0), stop=(d == n_dk - 1))
                junk_c = jpool.tile([P, P], f32, tag="junkg", name="junk_c")
                nc.vector.scalar_tensor_tensor(
                    out=junk_c[:], in0=g_ps[:], scalar=1.0, in1=ident[:],
                    op0=ALU.mult, op1=ALU.mult, accum_out=ssq[:, s:s + 1])
            nc.vector.tensor_scalar(out=ssq[:], in0=ssq[:], scalar1=1e-30,
                                    scalar2=None, op0=ALU.max)
            scale = spool.tile([P, n_sub], f32, name="scale")
            nc.scalar.activation(scale[:], ssq[:], AF.Ln)
            nc.scalar.activation(scale[:], scale[:], AF.Exp,
                                 bias=bias_lnS[:], scale=-0.5)  # S*rsqrt(ssq)

            for s in range(n_sub):
                ck = t * n_sub + s
                cos_ps = ps_c.tile([P, B], f32, tag="cos", name="cos_ps")
                for d in range(n_dk):
                    nc.tensor.matmul(cos_ps[:], wtt[d][:, s * P:(s + 1) * P],
                                     xnt[d][:], start=(d == 0),
                                     stop=(d == n_dk - 1))
                e_sb = epool.tile([P, B], bf16, tag="e", name="e_sb")
                nc.scalar.activation(e_sb[:], cos_ps[:], AF.Exp,
                                     bias=bias_m30[:], scale=scale[:, s:s + 1])
                nc.tensor.matmul(sumE_ps[:], ones_bf[:], e_sb[:],
                                 start=(ck == 0),
                                 stop=(ck == n_tiles * n_sub - 1))

        # ---- collective: AllReduce partial sums ----
        sumE_sb = sm.tile([1, B], f32)
        nc.vector.tensor_copy(sumE_sb[:], sumE_ps[:])
        cc_in = dram.tile([1, B], f32)
        cc_out = dram.tile([1, B], f32)
        nc.sync.dma_start(out=cc_in[:], in_=sumE_sb[:])
        nc.gpsimd.collective_compute(
            "AllReduce", ALU.add,
            replica_groups=[list(range(n_cores))],
            ins=[cc_in.opt()], outs=[cc_out.opt()])
        red = sm.tile([1, B], f32)
        nc.sync.dma_start(out=red[:], in_=cc_out[:])

        # ---- tail: flip [1,B] -> [128, n_bk], nll, mean ----
        fl_ps = ps_g.tile([P, n_bk], f32, tag="g", name="fl_ps")
        for k in range(n_bk):
            nc.tensor.transpose(fl_ps[:, k:k + 1], red[0:1, k * P:(k + 1) * P],
                                ident[0:1, 0:1])
        r0b = sm.tile([P, n_bk], f32)
        nc.vector.tensor_copy(r0b[:], fl_ps[:])
        total = sm.tile([P, n_bk], f32)
        nc.vector.scalar_tensor_tensor(out=total[:], in0=r0b[:],
                                       scalar=-pad_corr, in1=delta[:],
                                       op0=ALU.add, op1=ALU.add)
        lnt = sm.tile([P, n_bk], f32)
        nc.scalar.activation(lnt[:], total[:], AF.Ln)
        nc.vector.tensor_scalar(out=lnt[:], in0=lnt[:], scalar1=-BIAS,
                                scalar2=None, op0=ALU.add)
        nll = sm.tile([P, n_bk], f32)
        nc.vector.scalar_tensor_tensor(out=nll[:], in0=phi_f[:], scalar=-S,
                                       in1=lnt[:], op0=ALU.mult, op1=ALU.add)
        nll1 = sm.tile([P, 1], f32)
        nc.vector.reduce_sum(nll1[:], nll[:], axis=AX.X)
        mean_ps = ps_g.tile([1, 1], f32, tag="g", name="mean_ps")
        nc.tensor.matmul(mean_ps[:], ones_f[:], nll1[:], start=True, stop=True)
        mean_sb = sm.tile([1, 1], f32)
        nc.vector.tensor_scalar(out=mean_sb[:], in0=mean_ps[:],
                                scalar1=1.0 / float(B), scalar2=None,
                                op0=ALU.mult)
        nc.sync.dma_start(out=out_ext.ap()[:, :], in_=mean_sb[:])

    nc.compile()
    return nc


def _shard_inputs(input, weight, target, c_pad=C_PAD, c_real=C_SHARD,
                  n_cores=N_CORES):
    """Host-side data layout only: shard, pad, transpose, gather."""
    x = np.ascontiguousarray(input, dtype=np.float32)
    w = np.asarray(weight, dtype=np.float32)
    tgt = np.asarray(target).astype(np.int64)
    wtg = np.ascontiguousarray(w[tgt])  # [B, D] gathered target rows
    in_maps = []
    for j in range(n_cores):
        shard = w[j * c_real:(j + 1) * c_real]          # [c_real, D]
        wt = np.zeros((D, c_pad), dtype=np.float32)
        wt[:, :c_real] = shard.T
        in_maps.append({"wt": wt, "x": x, "wtg": wtg})
    return in_maps


_NC_CACHE = {}


def kernel(input, weight, target, _trace=False, _trace_kwargs=None):
    key = "full"
    if key not in _NC_CACHE:
        _NC_CACHE[key] = build_arcface_nc()
    nc = _NC_CACHE[key]
    in_maps = _shard_inputs(input, weight, target)
    res = run_bass_kernel_spmd(nc, in_maps, core_ids=list(range(N_CORES)),
                               trace=_trace, **(_trace_kwargs or {}))
    out = np.float32(res.results[0]["out"][0, 0])
    kernel.last_results = res
    return np.asarray(out, dtype=np.float32).reshape(())


if __name__ == "__main__":
    # quick self-check with random data (no reference import — kernel.py must
    # stay self-contained; see test.py for the real check)
    rng = np.random.default_rng(0)
    x = rng.standard_normal((B, D)).astype(np.float32)
    w = rng.standard_normal((NCLASS, D)).astype(np.float32) * 0.01
    t = rng.integers(0, NCLASS, size=(B,)).astype(np.int64)
    print("out:", kernel(x, w, t))


# revision 3
# speedup vs baseline: 1.3080x; 1.3080x over previous
"""ArcFace loss distributed Bass kernel for 8 TRN2 NeuronCores.

Strategy (class-parallel / tensor-parallel over the 100000-class dim):
  - Host: pad classes 100000 -> 8*12544, transpose W shard to [D, C_shard] per core,
    gather W[target] rows (pure data movement; no arithmetic on host).
  - Device (SPMD, identical program on 8 cores):
      * normalize x rows, transpose to xnT [D, B] (PE transpose)
      * stream WT tiles; f32r matmul cosT[c,b] = WT.T @ xnT (classes on partitions)
      * per-class ||w||^2 via bf16 gram matmul diag (diag extracted with one
        scalar_tensor_tensor against an identity mask, accum_out -> [c,1])
      * exp fused on ScalarE: E = exp(S*rsqrt(ssq)*P - 30) with per-partition
        scale AP (rsqrt computed as exp(-0.5*ln(ssq)+ln(S)) to stay in the
        natural_log_exp table set)
      * partition-sum of E via ones-vector matmul accumulating into one PSUM row
      * target-logit correction computed densely for all 512 rows on every core
        from host-gathered W[target] (row-dots on VectorE, margin math, tiny)
      * AllReduce the [1,512] partial sums across the 8 cores, then
        nll = 30 + ln(total) - S*phi, mean -> scalar
Fixed max shift of S=30 is used for the softmax (cos <= 1), so no running max
is needed: exp(S*cos - 30) is exact and never overflows/underflows in fp32.
"""

import math
from contextlib import ExitStack

import numpy as np

from concourse import bacc, masks, mybir, tile
from concourse.bass_utils import run_bass_kernel_spmd

N_CORES = 8
B = 512
D = 512
NCLASS = 100000
C_SHARD = NCLASS // N_CORES      # 12500
C_PAD = 12544                    # 98 * 128
C_TILE = 1792                    # 7 macro tiles of 14 subchunks
S = 30.0
MARGIN = 0.5
COS_M = math.cos(MARGIN)
SIN_M = math.sin(MARGIN)
BIAS = -30.0                     # fixed log-sum-exp shift (= -S)

f32 = mybir.dt.float32
f32r = mybir.dt.float32r
bf16 = mybir.dt.bfloat16
AF = mybir.ActivationFunctionType
ALU = mybir.AluOpType
AX = mybir.AxisListType

P = 128



def _pin_act_tables():
    """Force Exp and Ln onto the single natural_log_exp_and_others table set
    so walrus doesn't ping-pong ACT table loads between exp/ln sets."""
    import concourse.bacc as _bacc
    import concourse.hw_specs as _hw
    if getattr(_bacc, "_act_tables_pinned", False):
        return
    _orig = _hw.get_activation_tables

    def _pinned(arch):
        tabs = _orig(arch)
        both = {AF.Exp, AF.Ln}
        for name, fns in tabs.items():
            if name != "natural_log_exp_and_others":
                tabs[name] = fns - both
        return tabs

    _bacc.get_activation_tables = _pinned
    _bacc._act_tables_pinned = True


def build_arcface_nc(c_pad=C_PAD, c_tile=C_TILE, c_real=C_SHARD, n_cores=N_CORES):
    assert c_pad % c_tile == 0 and c_tile % P == 0
    n_tiles = c_pad // c_tile
    n_sub = c_tile // P
    n_dk = D // P
    n_bk = B // P
    pad_corr = float(n_cores * (c_pad - c_real)) * math.exp(BIAS)

    _pin_act_tables()
    nc = bacc.Bacc("TRN2", target_bir_lowering=False, debug=False,
                   num_devices=n_cores)

    wt_ext = nc.dram_tensor("wt", [D, c_pad], f32, kind="ExternalInput")
    x_ext = nc.dram_tensor("x", [B, D], f32, kind="ExternalInput")
    wtg_ext = nc.dram_tensor("wtg", [B, D], f32, kind="ExternalInput")
    out_ext = nc.dram_tensor("out", [1, 1], f32, kind="ExternalOutput")

    with ExitStack() as ctx:
        tc = ctx.enter_context(tile.TileContext(nc))
        cpool = ctx.enter_context(tc.tile_pool(name="consts", bufs=1))
        xpool = ctx.enter_context(tc.tile_pool(name="xpool", bufs=1))
        sm = ctx.enter_context(tc.tile_pool(name="smalls", bufs=1))
        spool = ctx.enter_context(tc.tile_pool(name="spool", bufs=2))
        wtpool = ctx.enter_context(tc.tile_pool(name="wtpool", bufs=2))
        wbpool = ctx.enter_context(tc.tile_pool(name="wbpool", bufs=2))
        epool = ctx.enter_context(tc.tile_pool(name="epool", bufs=4))
        jpool = ctx.enter_context(tc.tile_pool(name="jpool", bufs=2))
        ps_c = ctx.enter_context(tc.tile_pool(name="ps_c", bufs=3, space="PSUM"))
        ps_g = ctx.enter_context(tc.tile_pool(name="ps_g", bufs=2, space="PSUM"))
        ps_acc = ctx.enter_context(tc.tile_pool(name="ps_acc", bufs=1, space="PSUM"))
        dram = ctx.enter_context(tc.tile_pool(name="dram", bufs=1, space="DRAM"))

        # ---- constants ----
        ident = cpool.tile([P, P], f32)
        masks.make_identity(nc, ident[:])
        ones_bf = cpool.tile([P, 1], bf16)
        nc.vector.memset(ones_bf[:], 1.0)
        ones_f = cpool.tile([P, 1], f32)
        nc.vector.memset(ones_f[:], 1.0)
        bias_m30 = cpool.tile([P, 1], f32)
        nc.vector.memset(bias_m30[:], BIAS)
        bias_lnS = cpool.tile([P, 1], f32)
        nc.vector.memset(bias_lnS[:], float(np.log(S)))

        # ---- x prep: load, row-norms, normalize, transpose ----
        xb = [xpool.tile([P, D], f32, name=f"xb{k}") for k in range(n_bk)]
        for k in range(n_bk):
            nc.sync.dma_start(out=xb[k][:], in_=x_ext.ap()[k * P:(k + 1) * P, :])
        qx = sm.tile([P, n_bk], f32)
        junk_a = jpool.tile([P, D], f32, tag="junk")
        for k in range(n_bk):
            nc.vector.scalar_tensor_tensor(
                out=junk_a[:], in0=xb[k][:], scalar=1.0, in1=xb[k][:],
                op0=ALU.mult, op1=ALU.mult, accum_out=qx[:, k:k + 1])
        rx = sm.tile([P, n_bk], f32)
        nc.scalar.activation(rx[:], qx[:], AF.Ln)
        nc.scalar.activation(rx[:], rx[:], AF.Exp, scale=-0.5)  # rsqrt(qx)

        xn = [xpool.tile([P, D], f32, name=f"xn{k}") for k in range(n_bk)]
        for k in range(n_bk):
            nc.vector.tensor_scalar(out=xn[k][:], in0=xb[k][:],
                                    scalar1=rx[:, k:k + 1], scalar2=None,
                                    op0=ALU.mult)
        xnt = [xpool.tile([P, B], bf16, name=f"xnt{d}") for d in range(n_dk)]
        for k in range(n_bk):
            tp_ps = ps_c.tile([P, B], f32, tag="cos", name=f"tp_ps{k}")
            for d in range(n_dk):
                nc.tensor.transpose(tp_ps[:, d * P:(d + 1) * P],
                                    xn[k][:, d * P:(d + 1) * P], ident[:])
            for d in range(n_dk):
                nc.vector.tensor_copy(xnt[d][:, k * P:(k + 1) * P],
                                      tp_ps[:, d * P:(d + 1) * P])

        # ---- target margin terms (dense over all B rows, every core) ----
        wg = [xpool.tile([P, D], f32, name=f"wg{k}") for k in range(n_bk)]
        for k in range(n_bk):
            nc.sync.dma_start(out=wg[k][:], in_=wtg_ext.ap()[k * P:(k + 1) * P, :])
        qw = sm.tile([P, n_bk], f32)
        pt = sm.tile([P, n_bk], f32)
        junk_b = jpool.tile([P, D], f32, tag="junk")
        for k in range(n_bk):
            nc.vector.scalar_tensor_tensor(
                out=junk_b[:], in0=wg[k][:], scalar=1.0, in1=wg[k][:],
                op0=ALU.mult, op1=ALU.mult, accum_out=qw[:, k:k + 1])
            nc.vector.scalar_tensor_tensor(
                out=junk_b[:], in0=xb[k][:], scalar=1.0, in1=wg[k][:],
                op0=ALU.mult, op1=ALU.mult, accum_out=pt[:, k:k + 1])
        q = sm.tile([P, n_bk], f32)
        nc.vector.tensor_mul(q[:], qw[:], qx[:])
        nc.vector.tensor_scalar(out=q[:], in0=q[:], scalar1=1e-30, scalar2=None,
                                op0=ALU.max)
        rq = sm.tile([P, n_bk], f32)
        nc.scalar.activation(rq[:], q[:], AF.Ln)
        nc.scalar.activation(rq[:], rq[:], AF.Exp, scale=-0.5)
        cos_t = sm.tile([P, n_bk], f32)
        nc.vector.tensor_mul(cos_t[:], pt[:], rq[:])
        # sine = sqrt(max(1 - cos^2, eps))
        om = sm.tile([P, n_bk], f32)
        nc.vector.tensor_mul(om[:], cos_t[:], cos_t[:])
        nc.vector.tensor_scalar(out=om[:], in0=om[:], scalar1=-1.0, scalar2=1.0,
                                op0=ALU.mult, op1=ALU.add)
        nc.vector.tensor_scalar(out=om[:], in0=om[:], scalar1=1e-36, scalar2=None,
                                op0=ALU.max)
        sine = sm.tile([P, n_bk], f32)
        nc.scalar.activation(sine[:], om[:], AF.Ln)
        nc.scalar.activation(sine[:], sine[:], AF.Exp, scale=0.5)
        # phi = cos*COS_M - sine*SIN_M ; easy margin: cos>0 ? phi : cos
        tmp = sm.tile([P, n_bk], f32)
        nc.vector.tensor_scalar(out=tmp[:], in0=cos_t[:], scalar1=COS_M,
                                scalar2=None, op0=ALU.mult)
        phi = sm.tile([P, n_bk], f32)
        nc.vector.scalar_tensor_tensor(out=phi[:], in0=sine[:], scalar=-SIN_M,
                                       in1=tmp[:], op0=ALU.mult, op1=ALU.add)
        mask = sm.tile([P, n_bk], mybir.dt.uint8)
        nc.vector.tensor_scalar(out=mask[:], in0=cos_t[:], scalar1=0.0,
                                scalar2=None, op0=ALU.is_gt)
        phi_f = sm.tile([P, n_bk], f32)
        nc.vector.select(phi_f[:], mask[:], phi[:], cos_t[:])
        # delta = exp(S*phi_f - 30) - exp(S*cos_t - 30)
        e1 = sm.tile([P, n_bk], f32)
        nc.scalar.activation(e1[:], phi_f[:], AF.Exp, bias=bias_m30[:], scale=S)
        e2 = sm.tile([P, n_bk], f32)
        nc.scalar.activation(e2[:], cos_t[:], AF.Exp, bias=bias_m30[:], scale=S)
        delta = sm.tile([P, n_bk], f32)
        nc.vector.tensor_sub(delta[:], e1[:], e2[:])

        # ---- main loop over class tiles ----
        sumE_ps = ps_acc.tile([1, B], f32)
        for t in range(n_tiles):
            wtt = [wtpool.tile([P, c_tile], f32, name=f"wtt{d}", tag=f"wtt{d}")
                   for d in range(n_dk)]
            for d in range(n_dk):
                nc.sync.dma_start(
                    out=wtt[d][:],
                    in_=wt_ext.ap()[d * P:(d + 1) * P,
                                    t * c_tile:(t + 1) * c_tile])
            wtb = [wbpool.tile([P, c_tile], bf16, name=f"wtb{d}", tag=f"wtb{d}")
                   for d in range(n_dk)]
            for d in range(n_dk):
                nc.vector.tensor_copy(wtb[d][:], wtt[d][:])

            ssq = spool.tile([P, n_sub], f32, name="ssq")
            for s in range(n_sub):
                g_ps = ps_g.tile([P, P], f32, tag="g", name="g_ps")
                for d in range(n_dk):
                    nc.tensor.matmul(g_ps[:], wtb[d][:, s * P:(s + 1) * P],
                                     wtb[d][:, s * P:(s + 1) * P],
                                     start=(d == 0), stop=(d == n_dk - 1))
                junk_c = jpool.tile([P, P], f32, tag="junkg", name="junk_c")
                nc.vector.scalar_tensor_tensor(
                    out=junk_c[:], in0=g_ps[:], scalar=1.0, in1=ident[:],
                    op0=ALU.mult, op1=ALU.mult, accum_out=ssq[:, s:s + 1])
            nc.vector.tensor_scalar(out=ssq[:], in0=ssq[:], scalar1=1e-30,
                                    scalar2=None, op0=ALU.max)
            scale = spool.tile([P, n_sub], f32, name="scale")
            nc.scalar.activation(scale[:], ssq[:], AF.Ln)
            nc.scalar.activation(scale[:], scale[:], AF.Exp,
                                 bias=bias_lnS[:], scale=-0.5)  # S*rsqrt(ssq)

            for s in range(n_sub):
                ck = t * n_sub + s
                cos_ps = ps_c.tile([P, B], f32, tag="cos", name="cos_ps")
                for d in range(n_dk):
                    nc.tensor.matmul(cos_ps[:], wtb[d][:, s * P:(s + 1) * P],
                                     xnt[d][:], start=(d == 0),
                                     stop=(d == n_dk - 1))
                e_sb = epool.tile([P, B], bf16, tag="e", name="e_sb")
                nc.scalar.activation(e_sb[:], cos_ps[:], AF.Exp,
                                     bias=bias_m30[:], scale=scale[:, s:s + 1])
                nc.tensor.matmul(sumE_ps[:], ones_bf[:], e_sb[:],
                                 start=(ck == 0),
                                 stop=(ck == n_tiles * n_sub - 1))

        # ---- collective: AllReduce partial sums ----
        sumE_sb = sm.tile([1, B], f32)
        nc.vector.tensor_copy(sumE_sb[:], sumE_ps[:])
        cc_in = dram.tile([1, B], f32)
        cc_out = dram.tile([1, B], f32)
        nc.sync.dma_start(out=cc_in[:], in_=sumE_sb[:])
        nc.gpsimd.collective_compute(
            "AllReduce", ALU.add,
            replica_groups=[list(range(n_cores))],
            ins=[cc_in.opt()], outs=[cc_out.opt()])
        red = sm.tile([1, B], f32)
        nc.sync.dma_start(out=red[:], in_=cc_out[:])

        # ---- tail: flip [1,B] -> [128, n_bk], nll, mean ----
        fl_ps = ps_g.tile([P, n_bk], f32, tag="g", name="fl_ps")
        for k in range(n_bk):
            nc.tensor.transpose(fl_ps[:, k:k + 1], red[0:1, k * P:(k + 1) * P],
                                ident[0:1, 0:1])
        r0b = sm.tile([P, n_bk], f32)
        nc.vector.tensor_copy(r0b[:], fl_ps[:])
        total = sm.tile([P, n_bk], f32)
        nc.vector.scalar_tensor_tensor(out=total[:], in0=r0b[:],
                                       scalar=-pad_corr, in1=delta[:],
                                       op0=ALU.add, op1=ALU.add)
        lnt = sm.tile([P, n_bk], f32)
        nc.scalar.activation(lnt[:], total[:], AF.Ln)
        nc.vector.tensor_scalar(out=lnt[:], in0=lnt[:], scalar1=-BIAS,
                                scalar2=None, op0=ALU.add)
        nll = sm.tile([P, n_bk], f32)
        nc.vector.scalar_tensor_tensor(out=nll[:], in0=phi_f[:], scalar=-S,
                                       in1=lnt[:], op0=ALU.mult, op1=ALU.add)
        nll1 = sm.tile([P, 1], f32)
        nc.vector.reduce_sum(nll1[:], nll[:], axis=AX.X)
        mean_ps = ps_g.tile([1, 1], f32, tag="g", name="mean_ps")
        nc.tensor.matmul(mean_ps[:], ones_f[:], nll1[:], start=True, stop=True)
        mean_sb = sm.tile([1, 1], f32)
        nc.vector.tensor_scalar(out=mean_sb[:], in0=mean_ps[:],
                                scalar1=1.0 / float(B), scalar2=None,
                                op0=ALU.mult)
        nc.sync.dma_start(out=out_ext.ap()[:, :], in_=mean_sb[:])

    nc.compile()
    return nc


def _shard_inputs(input, weight, target, c_pad=C_PAD, c_real=C_SHARD,
                  n_cores=N_CORES):
    """Host-side data layout only: shard, pad, transpose, gather."""
    x = np.ascontiguousarray(input, dtype=np.float32)
    w = np.asarray(weight, dtype=np.float32)
    tgt = np.asarray(target).astype(np.int64)
    wtg = np.ascontiguousarray(w[tgt])  # [B, D] gathered target rows
    in_maps = []
    for j in range(n_cores):
        shard = w[j * c_real:(j + 1) * c_real]          # [c_real, D]
        wt = np.zeros((D, c_pad), dtype=np.float32)
        wt[:, :c_real] = shard.T
        in_maps.append({"wt": wt, "x": x, "wtg": wtg})
    return in_maps


_NC_CACHE = {}


def kernel(input, weight, target, _trace=False, _trace_kwargs=None):
    key = "full"
    if key not in _NC_CACHE:
        _NC_CACHE[key] = build_arcface_nc()
    nc = _NC_CACHE[key]
    in_maps = _shard_inputs(input, weight, target)
    res = run_bass_kernel_spmd(nc, in_maps, core_ids=list(range(N_CORES)),
                               trace=_trace, **(_trace_kwargs or {}))
    out = np.float32(res.results[0]["out"][0, 0])
    kernel.last_results = res
    return np.asarray(out, dtype=np.float32).reshape(())


if __name__ == "__main__":
    # quick self-check with random data (no reference import — kernel.py must
    # stay self-contained; see test.py for the real check)
    rng = np.random.default_rng(0)
    x = rng.standard_normal((B, D)).astype(np.float32)
    w = rng.standard_normal((NCLASS, D)).astype(np.float32) * 0.01
    t = rng.integers(0, NCLASS, size=(B,)).astype(np.int64)
    print("out:", kernel(x, w, t))


# revision 4
# speedup vs baseline: 1.3973x; 1.0683x over previous
"""ArcFace loss distributed Bass kernel for 8 TRN2 NeuronCores.

Strategy (class-parallel / tensor-parallel over the 100000-class dim):
  - Host: pad classes 100000 -> 8*12544, transpose W shard to [D, C_shard] per core,
    gather W[target] rows (pure data movement; no arithmetic on host).
  - Device (SPMD, identical program on 8 cores):
      * normalize x rows, transpose to xnT [D, B] (PE transpose)
      * stream WT tiles; f32r matmul cosT[c,b] = WT.T @ xnT (classes on partitions)
      * per-class ||w||^2 via bf16 gram matmul diag (diag extracted with one
        scalar_tensor_tensor against an identity mask, accum_out -> [c,1])
      * exp fused on ScalarE: E = exp(S*rsqrt(ssq)*P - 30) with per-partition
        scale AP (rsqrt computed as exp(-0.5*ln(ssq)+ln(S)) to stay in the
        natural_log_exp table set)
      * partition-sum of E via ones-vector matmul accumulating into one PSUM row
      * target-logit correction computed densely for all 512 rows on every core
        from host-gathered W[target] (row-dots on VectorE, margin math, tiny)
      * AllReduce the [1,512] partial sums across the 8 cores, then
        nll = 30 + ln(total) - S*phi, mean -> scalar
Fixed max shift of S=30 is used for the softmax (cos <= 1), so no running max
is needed: exp(S*cos - 30) is exact and never overflows/underflows in fp32.
"""

import math
from contextlib import ExitStack

import numpy as np

from concourse import bacc, masks, mybir, tile
from concourse.bass_utils import run_bass_kernel_spmd

N_CORES = 8
B = 512
D = 512
NCLASS = 100000
C_SHARD = NCLASS // N_CORES      # 12500
C_PAD = 12544                    # 98 * 128
C_TILE = 1792                    # 7 macro tiles of 14 subchunks
S = 30.0
MARGIN = 0.5
COS_M = math.cos(MARGIN)
SIN_M = math.sin(MARGIN)
BIAS = -30.0                     # fixed log-sum-exp shift (= -S)

f32 = mybir.dt.float32
f32r = mybir.dt.float32r
bf16 = mybir.dt.bfloat16
AF = mybir.ActivationFunctionType
ALU = mybir.AluOpType
AX = mybir.AxisListType

P = 128



def _pin_act_tables():
    """Force Exp and Ln onto the single natural_log_exp_and_others table set
    so walrus doesn't ping-pong ACT table loads between exp/ln sets."""
    import concourse.bacc as _bacc
    import concourse.hw_specs as _hw
    if getattr(_bacc, "_act_tables_pinned", False):
        return
    _orig = _hw.get_activation_tables

    def _pinned(arch):
        tabs = _orig(arch)
        both = {AF.Exp, AF.Ln}
        for name, fns in tabs.items():
            if name != "natural_log_exp_and_others":
                tabs[name] = fns - both
        return tabs

    _bacc.get_activation_tables = _pinned
    _bacc._act_tables_pinned = True


def build_arcface_nc(c_pad=C_PAD, c_tile=C_TILE, c_real=C_SHARD, n_cores=N_CORES):
    assert c_pad % c_tile == 0 and c_tile % P == 0
    n_tiles = c_pad // c_tile
    n_sub = c_tile // P
    n_dk = D // P
    n_bk = B // P
    pad_corr = float(n_cores * (c_pad - c_real)) * math.exp(BIAS)

    _pin_act_tables()
    nc = bacc.Bacc("TRN2", target_bir_lowering=False, debug=False,
                   num_devices=n_cores)

    wt_ext = nc.dram_tensor("wt", [D, c_pad], f32, kind="ExternalInput")
    x_ext = nc.dram_tensor("x", [B, D], f32, kind="ExternalInput")
    wtg_ext = nc.dram_tensor("wtg", [B, D], f32, kind="ExternalInput")
    out_ext = nc.dram_tensor("out", [1, 1], f32, kind="ExternalOutput")

    with ExitStack() as ctx:
        tc = ctx.enter_context(tile.TileContext(nc))
        cpool = ctx.enter_context(tc.tile_pool(name="consts", bufs=1))
        xpool = ctx.enter_context(tc.tile_pool(name="xpool", bufs=1))
        sm = ctx.enter_context(tc.tile_pool(name="smalls", bufs=1))
        spool = ctx.enter_context(tc.tile_pool(name="spool", bufs=2))
        wtpool = ctx.enter_context(tc.tile_pool(name="wtpool", bufs=2))
        wbpool = ctx.enter_context(tc.tile_pool(name="wbpool", bufs=2))
        epool = ctx.enter_context(tc.tile_pool(name="epool", bufs=6))
        jpool = ctx.enter_context(tc.tile_pool(name="jpool", bufs=2))
        ps_c = ctx.enter_context(tc.tile_pool(name="ps_c", bufs=3, space="PSUM"))
        ps_g = ctx.enter_context(tc.tile_pool(name="ps_g", bufs=4, space="PSUM"))
        ps_acc = ctx.enter_context(tc.tile_pool(name="ps_acc", bufs=1, space="PSUM"))
        dram = ctx.enter_context(tc.tile_pool(name="dram", bufs=1, space="DRAM"))

        # ---- constants ----
        ident = cpool.tile([P, P], f32)
        masks.make_identity(nc, ident[:])
        ones_bf = cpool.tile([P, 1], bf16)
        nc.vector.memset(ones_bf[:], 1.0)
        ones_f = cpool.tile([P, 1], f32)
        nc.vector.memset(ones_f[:], 1.0)
        bias_m30 = cpool.tile([P, 1], f32)
        nc.vector.memset(bias_m30[:], BIAS)
        bias_lnS = cpool.tile([P, 1], f32)
        nc.vector.memset(bias_lnS[:], float(np.log(S)))

        # ---- x prep: load, row-norms, normalize, transpose ----
        xb = [xpool.tile([P, D], f32, name=f"xb{k}") for k in range(n_bk)]
        for k in range(n_bk):
            nc.gpsimd.dma_start(out=xb[k][:], in_=x_ext.ap()[k * P:(k + 1) * P, :])
        qx = sm.tile([P, n_bk], f32)
        junk_a = jpool.tile([P, D], f32, tag="junk")
        for k in range(n_bk):
            nc.vector.scalar_tensor_tensor(
                out=junk_a[:], in0=xb[k][:], scalar=1.0, in1=xb[k][:],
                op0=ALU.mult, op1=ALU.mult, accum_out=qx[:, k:k + 1])
        rx = sm.tile([P, n_bk], f32)
        nc.scalar.activation(rx[:], qx[:], AF.Ln)
        nc.scalar.activation(rx[:], rx[:], AF.Exp, scale=-0.5)  # rsqrt(qx)

        xn = [xpool.tile([P, D], f32, name=f"xn{k}") for k in range(n_bk)]
        for k in range(n_bk):
            nc.vector.tensor_scalar(out=xn[k][:], in0=xb[k][:],
                                    scalar1=rx[:, k:k + 1], scalar2=None,
                                    op0=ALU.mult)
        xnt = [xpool.tile([P, B], bf16, name=f"xnt{d}") for d in range(n_dk)]
        for k in range(n_bk):
            tp_ps = ps_c.tile([P, B], f32, tag="cos", name=f"tp_ps{k}")
            for d in range(n_dk):
                nc.tensor.transpose(tp_ps[:, d * P:(d + 1) * P],
                                    xn[k][:, d * P:(d + 1) * P], ident[:])
            for d in range(n_dk):
                nc.vector.tensor_copy(xnt[d][:, k * P:(k + 1) * P],
                                      tp_ps[:, d * P:(d + 1) * P])

        # ---- target margin terms (dense over all B rows, every core) ----
        wg = [xpool.tile([P, D], f32, name=f"wg{k}") for k in range(n_bk)]
        for k in range(n_bk):
            nc.gpsimd.dma_start(out=wg[k][:], in_=wtg_ext.ap()[k * P:(k + 1) * P, :])
        qw = sm.tile([P, n_bk], f32)
        pt = sm.tile([P, n_bk], f32)
        junk_b = jpool.tile([P, D], f32, tag="junk")
        for k in range(n_bk):
            nc.vector.scalar_tensor_tensor(
                out=junk_b[:], in0=wg[k][:], scalar=1.0, in1=wg[k][:],
                op0=ALU.mult, op1=ALU.mult, accum_out=qw[:, k:k + 1])
            nc.vector.scalar_tensor_tensor(
                out=junk_b[:], in0=xb[k][:], scalar=1.0, in1=wg[k][:],
                op0=ALU.mult, op1=ALU.mult, accum_out=pt[:, k:k + 1])
        q = sm.tile([P, n_bk], f32)
        nc.vector.tensor_mul(q[:], qw[:], qx[:])
        nc.vector.tensor_scalar(out=q[:], in0=q[:], scalar1=1e-30, scalar2=None,
                                op0=ALU.max)
        rq = sm.tile([P, n_bk], f32)
        nc.scalar.activation(rq[:], q[:], AF.Ln)
        nc.scalar.activation(rq[:], rq[:], AF.Exp, scale=-0.5)
        cos_t = sm.tile([P, n_bk], f32)
        nc.vector.tensor_mul(cos_t[:], pt[:], rq[:])
        # sine = sqrt(max(1 - cos^2, eps))
        om = sm.tile([P, n_bk], f32)
        nc.vector.tensor_mul(om[:], cos_t[:], cos_t[:])
        nc.vector.tensor_scalar(out=om[:], in0=om[:], scalar1=-1.0, scalar2=1.0,
                                op0=ALU.mult, op1=ALU.add)
        nc.vector.tensor_scalar(out=om[:], in0=om[:], scalar1=1e-36, scalar2=None,
                                op0=ALU.max)
        sine = sm.tile([P, n_bk], f32)
        nc.scalar.activation(sine[:], om[:], AF.Ln)
        nc.scalar.activation(sine[:], sine[:], AF.Exp, scale=0.5)
        # phi = cos*COS_M - sine*SIN_M ; easy margin: cos>0 ? phi : cos
        tmp = sm.tile([P, n_bk], f32)
        nc.vector.tensor_scalar(out=tmp[:], in0=cos_t[:], scalar1=COS_M,
                                scalar2=None, op0=ALU.mult)
        phi = sm.tile([P, n_bk], f32)
        nc.vector.scalar_tensor_tensor(out=phi[:], in0=sine[:], scalar=-SIN_M,
                                       in1=tmp[:], op0=ALU.mult, op1=ALU.add)
        mask = sm.tile([P, n_bk], mybir.dt.uint8)
        nc.vector.tensor_scalar(out=mask[:], in0=cos_t[:], scalar1=0.0,
                                scalar2=None, op0=ALU.is_gt)
        phi_f = sm.tile([P, n_bk], f32)
        nc.vector.select(phi_f[:], mask[:], phi[:], cos_t[:])
        # delta = exp(S*phi_f - 30) - exp(S*cos_t - 30)
        e1 = sm.tile([P, n_bk], f32)
        nc.scalar.activation(e1[:], phi_f[:], AF.Exp, bias=bias_m30[:], scale=S)
        e2 = sm.tile([P, n_bk], f32)
        nc.scalar.activation(e2[:], cos_t[:], AF.Exp, bias=bias_m30[:], scale=S)
        delta = sm.tile([P, n_bk], f32)
        nc.vector.tensor_sub(delta[:], e1[:], e2[:])

        # ---- main loop over class tiles ----
        sumE_ps = ps_acc.tile([1, B], f32)
        for t in range(n_tiles):
            wtt = [wtpool.tile([P, c_tile], f32, name=f"wtt{d}", tag=f"wtt{d}")
                   for d in range(n_dk)]
            for d in range(n_dk):
                nc.sync.dma_start(
                    out=wtt[d][:],
                    in_=wt_ext.ap()[d * P:(d + 1) * P,
                                    t * c_tile:(t + 1) * c_tile])
            wtb = [wbpool.tile([P, c_tile], bf16, name=f"wtb{d}", tag=f"wtb{d}")
                   for d in range(n_dk)]
            for d in range(n_dk):
                nc.vector.tensor_copy(wtb[d][:], wtt[d][:])

            ssq = spool.tile([P, n_sub], f32, name="ssq")
            for s in range(n_sub):
                g_ps = ps_g.tile([P, P], f32, tag="g", name="g_ps")
                for d in range(n_dk):
                    nc.tensor.matmul(g_ps[:], wtb[d][:, s * P:(s + 1) * P],
                                     wtb[d][:, s * P:(s + 1) * P],
                                     start=(d == 0), stop=(d == n_dk - 1))
                junk_c = jpool.tile([P, P], f32, tag="junkg", name="junk_c")
                nc.vector.scalar_tensor_tensor(
                    out=junk_c[:], in0=g_ps[:], scalar=1.0, in1=ident[:],
                    op0=ALU.mult, op1=ALU.mult, accum_out=ssq[:, s:s + 1])
            nc.vector.tensor_scalar(out=ssq[:], in0=ssq[:], scalar1=1e-30,
                                    scalar2=None, op0=ALU.max)
            scale = spool.tile([P, n_sub], f32, name="scale")
            nc.scalar.activation(scale[:], ssq[:], AF.Ln)
            nc.scalar.activation(scale[:], scale[:], AF.Exp,
                                 bias=bias_lnS[:], scale=-0.5)  # S*rsqrt(ssq)

            for s in range(n_sub):
                ck = t * n_sub + s
                cos_ps = ps_c.tile([P, B], f32, tag="cos", name="cos_ps")
                for d in range(n_dk):
                    nc.tensor.matmul(cos_ps[:], wtb[d][:, s * P:(s + 1) * P],
                                     xnt[d][:], start=(d == 0),
                                     stop=(d == n_dk - 1))
                e_sb = epool.tile([P, B], bf16, tag="e", name="e_sb")
                nc.scalar.activation(e_sb[:], cos_ps[:], AF.Exp,
                                     bias=bias_m30[:], scale=scale[:, s:s + 1])
                nc.tensor.matmul(sumE_ps[:], ones_bf[:], e_sb[:],
                                 start=(ck == 0),
                                 stop=(ck == n_tiles * n_sub - 1))

        # ---- collective: AllReduce partial sums ----
        sumE_sb = sm.tile([1, B], f32)
        nc.vector.tensor_copy(sumE_sb[:], sumE_ps[:])
        cc_in = dram.tile([1, B], f32)
        cc_out = dram.tile([1, B], f32)
        nc.sync.dma_start(out=cc_in[:], in_=sumE_sb[:])
        nc.gpsimd.collective_compute(
            "AllReduce", ALU.add,
            replica_groups=[list(range(n_cores))],
            ins=[cc_in.opt()], outs=[cc_out.opt()])
        red = sm.tile([1, B], f32)
        nc.sync.dma_start(out=red[:], in_=cc_out[:])

        # ---- tail: flip [1,B] -> [128, n_bk], nll, mean ----
        fl_ps = ps_g.tile([P, n_bk], f32, tag="g", name="fl_ps")
        for k in range(n_bk):
            nc.tensor.transpose(fl_ps[:, k:k + 1], red[0:1, k * P:(k + 1) * P],
                                ident[0:1, 0:1])
        r0b = sm.tile([P, n_bk], f32)
        nc.vector.tensor_copy(r0b[:], fl_ps[:])
        total = sm.tile([P, n_bk], f32)
        nc.vector.scalar_tensor_tensor(out=total[:], in0=r0b[:],
                                       scalar=-pad_corr, in1=delta[:],
                                       op0=ALU.add, op1=ALU.add)
        lnt = sm.tile([P, n_bk], f32)
        nc.scalar.activation(lnt[:], total[:], AF.Ln)
        nc.vector.tensor_scalar(out=lnt[:], in0=lnt[:], scalar1=-BIAS,
                                scalar2=None, op0=ALU.add)
        nll = sm.tile([P, n_bk], f32)
        nc.vector.scalar_tensor_tensor(out=nll[:], in0=phi_f[:], scalar=-S,
                                       in1=lnt[:], op0=ALU.mult, op1=ALU.add)
        nll1 = sm.tile([P, 1], f32)
        nc.vector.reduce_sum(nll1[:], nll[:], axis=AX.X)
        mean_ps = ps_g.tile([1, 1], f32, tag="g", name="mean_ps")
        nc.tensor.matmul(mean_ps[:], ones_f[:], nll1[:], start=True, stop=True)
        mean_sb = sm.tile([1, 1], f32)
        nc.vector.tensor_scalar(out=mean_sb[:], in0=mean_ps[:],
                                scalar1=1.0 / float(B), scalar2=None,
                                op0=ALU.mult)
        nc.sync.dma_start(out=out_ext.ap()[:, :], in_=mean_sb[:])

    nc.compile()
    return nc


def _shard_inputs(input, weight, target, c_pad=C_PAD, c_real=C_SHARD,
                  n_cores=N_CORES):
    """Host-side data layout only: shard, pad, transpose, gather."""
    x = np.ascontiguousarray(input, dtype=np.float32)
    w = np.asarray(weight, dtype=np.float32)
    tgt = np.asarray(target).astype(np.int64)
    wtg = np.ascontiguousarray(w[tgt])  # [B, D] gathered target rows
    in_maps = []
    for j in range(n_cores):
        shard = w[j * c_real:(j + 1) * c_real]          # [c_real, D]
        wt = np.zeros((D, c_pad), dtype=np.float32)
        wt[:, :c_real] = shard.T
        in_maps.append({"wt": wt, "x": x, "wtg": wtg})
    return in_maps


_NC_CACHE = {}


def kernel(input, weight, target, _trace=False, _trace_kwargs=None):
    key = "full"
    if key not in _NC_CACHE:
        _NC_CACHE[key] = build_arcface_nc()
    nc = _NC_CACHE[key]
    in_maps = _shard_inputs(input, weight, target)
    res = run_bass_kernel_spmd(nc, in_maps, core_ids=list(range(N_CORES)),
                               trace=_trace, **(_trace_kwargs or {}))
    out = np.float32(res.results[0]["out"][0, 0])
    kernel.last_results = res
    return np.asarray(out, dtype=np.float32).reshape(())


if __name__ == "__main__":
    # quick self-check with random data (no reference import — kernel.py must
    # stay self-contained; see test.py for the real check)
    rng = np.random.default_rng(0)
    x = rng.standard_normal((B, D)).astype(np.float32)
    w = rng.standard_normal((NCLASS, D)).astype(np.float32) * 0.01
    t = rng.integers(0, NCLASS, size=(B,)).astype(np.int64)
    print("out:", kernel(x, w, t))
